# revision 12
# baseline (speedup 1.0000x reference)
"""Trainium2 Bass kernel for nn_BiasingGateB (retrieval_knn, 8 NeuronCores).

Reference computation (for x:[64,2048,1024] f32):
    inp  = mean_T(x) @ W_p + b_p                        # [64,1024]
    sim  = cosine_sim(inp, patterns)                    # [64,64]
    gate = sigmoid(max_m sim)
    out  = where(gate > 0.8, bias_table[argmax_m sim], 0)   # [64,16]

Sharding: data-parallel over batch. Core c owns batches [8c, 8c+8).
The 512 MB x tensor dominates (memory regime): each core streams its
64 MB shard through SBUF in 16 x 4 MB chunks and reduces over T on the
TensorEngine (one-hot-column matmuls accumulating into one PSUM
[8,1024] tile), which hides the reduction entirely under the DMA
stream. W_p / patterns / bias_table are replicated. The projection,
normalization, similarity, argmax/gate and bias_table gather run as a
tiny on-device epilogue; the host only shards inputs and concatenates
the eight [8,16] outputs. xa is unused by the reference and never
touched.
"""

import numpy as np

B, T, D, H, M = 64, 2048, 1024, 16, 64
NCORES = 8
BPC = B // NCORES       # batches per core
KCH = 2                 # x-chunks per batch (4 MB each)
JD = T // (KCH * 128)   # T-rows per partition per chunk
DCH = D // 128          # 128-row chunks of the contraction dim
EPS = 1e-8
THRESHOLD = 0.8

_CACHE = {}


def build_bass():
    """Build (and cache) the per-core Bass module."""
    if "nc" in _CACHE:
        return _CACHE["nc"]

    import concourse.bacc as bacc
    import concourse.bass as bass
    import concourse.mybir as mybir
    import concourse.tile as tile
    from contextlib import ExitStack

    f32 = mybir.dt.float32
    f32r = mybir.dt.float32r
    bf16 = mybir.dt.bfloat16
    AF = mybir.ActivationFunctionType
    ALU = mybir.AluOpType
    AX = mybir.AxisListType
    PSUM = bass.MemorySpace.PSUM

    nc = bacc.Bacc("TRN2", target_bir_lowering=False, debug=False)

    x_d = nc.declare_dram_parameter("x", [BPC, KCH, 128, JD, D], f32r, isOutput=False)
    w_d = nc.declare_dram_parameter("W_p", [DCH, 128, D], f32, isOutput=False)
    bp_d = nc.declare_dram_parameter("b_p", [1, D], f32, isOutput=False)
    pat_d = nc.declare_dram_parameter("patterns", [M, D], f32, isOutput=False)
    bt_d = nc.declare_dram_parameter("bias_table", [M, H], f32, isOutput=False)
    oh_d = nc.declare_dram_parameter("oh", [128, BPC * BPC], f32r, isOutput=False)
    one_d = nc.declare_dram_parameter("ones1", [1, BPC], f32, isOutput=False)
    id_d = nc.declare_dram_parameter("ident", [128, 128], f32, isOutput=False)
    out_d = nc.declare_dram_parameter("out", [BPC, H], f32, isOutput=True)

    with tile.TileContext(nc) as tc, ExitStack() as ctx:
        xp = ctx.enter_context(tc.tile_pool(name="xp", bufs=4))
        cst = ctx.enter_context(tc.tile_pool(name="cst", bufs=1))
        sm = ctx.enter_context(tc.tile_pool(name="sm", bufs=1))
        ps_s = ctx.enter_context(tc.tile_pool(name="ps_s", bufs=1, space=PSUM))
        ps_p = ctx.enter_context(tc.tile_pool(name="ps_p", bufs=1, space=PSUM))
        ps_t = ctx.enter_context(tc.tile_pool(name="ps_t", bufs=2, space=PSUM))
        ps_g = ctx.enter_context(tc.tile_pool(name="ps_g", bufs=1, space=PSUM))

        # Small replicated inputs + constants (SWDGE queue, off the x stream)
        oh = cst.tile([128, BPC * BPC], f32r)
        nc.sync.dma_start(oh[:], oh_d[:])
        # (epilogue-only constants are loaded after the stream loop below)

        # ---- Phase 1: stream x, accumulate per-batch sums over T into PSUM.
        # lhsT = one-hot column block for batch b, so row b of s_ps gets
        # sum_k rhs[k,:] and every other row accumulates 0.
        s_ps = ps_s.tile([BPC, D], f32)
        first = (0, 0, 0)
        last = (BPC - 1, KCH - 1, JD - 1)
        # explicit queue schedule: sync/scalar take 24 MB each, gpsimd
        # (which also carries the ~5.6 MB of constants) takes 16 MB
        qsched = [nc.sync, nc.scalar, nc.gpsimd] * 5 + [nc.scalar]
        for b in range(BPC):
            for k in range(KCH):
                xt = xp.tile([128, JD, D], f32r)
                eng = qsched[b * KCH + k]
                if b == 0 and k == 0:
                    # split the first chunk into 1 MB pieces so the PE (and
                    # buffer-slot recycling) starts as early as possible
                    for s_ in range(4):
                        eng.dma_start(xt[:, 2 * s_:2 * s_ + 2, :],
                                      x_d[0, 0, :, 2 * s_:2 * s_ + 2, :])
                else:
                    eng.dma_start(xt[:], x_d[b, k])
                for j in range(JD):
                    for h in range(2):
                        nc.tensor.matmul(
                            s_ps[:, h * 512:(h + 1) * 512],
                            lhsT=oh[:, b * BPC:(b + 1) * BPC],
                            rhs=xt[:, j, h * 512:(h + 1) * 512],
                            start=((b, k, j) == first),
                            stop=((b, k, j) == last),
                        )

        ident = cst.tile([128, 128], f32)
        nc.gpsimd.dma_start(ident[:], id_d[:])
        ones1 = cst.tile([1, BPC], bf16)
        nc.gpsimd.dma_start(ones1[:], one_d[:])
        bp = cst.tile([1, D], bf16)
        nc.gpsimd.dma_start(bp[:], bp_d[:])
        pat = cst.tile([M, D], f32)
        nc.gpsimd.dma_start(pat[:], pat_d[:])
        bt = cst.tile([M, H], f32)
        nc.gpsimd.dma_start(bt[:], bt_d[:])
        # W_p cast to bf16 in the SWDGE transfer: halves its SBUF footprint
        # (frees room for a 4th x buffer) and single-pass PE matmuls
        wt = cst.tile([128, DCH, D], bf16)
        for c in range(DCH):
            nc.gpsimd.dma_start(wt[:, c, :], w_d[c])

        # ---- Phase 2 (tiny epilogue, all on-device) ----
        # mean over T
        inp = sm.tile([BPC, D], f32)
        nc.scalar.mul(inp[:], s_ps[:], 1.0 / T)

        # transpose inp -> inpT [128, DCH*BPC] (d-chunk c in cols [c*8,(c+1)*8))
        inpT = sm.tile([128, DCH * BPC], bf16)
        for c in range(DCH):
            tp = ps_t.tile([128, BPC], f32, tag="tp")
            nc.tensor.transpose(tp[:], inp[:, c * 128:(c + 1) * 128], ident[0:BPC, 0:BPC])
            nc.vector.tensor_copy(inpT[:, c * BPC:(c + 1) * BPC], tp[:])

        # proj = inp @ W_p + b_p  (bias folded in as a K=1 matmul)
        p_ps = ps_p.tile([BPC, D], f32)
        for h in range(2):
            for c in range(DCH):
                nc.tensor.matmul(
                    p_ps[:, h * 512:(h + 1) * 512],
                    lhsT=inpT[:, c * BPC:(c + 1) * BPC],
                    rhs=wt[:, c, h * 512:(h + 1) * 512],
                    start=(c == 0),
                    stop=False,
                )
            nc.tensor.matmul(
                p_ps[:, h * 512:(h + 1) * 512],
                lhsT=ones1[0:1, :],
                rhs=bp[0:1, h * 512:(h + 1) * 512],
                start=False,
                stop=True,
            )
        proj = sm.tile([BPC, D], f32)
        nc.scalar.copy(proj[:], p_ps[:])

        # row norms of proj -> inv_inp = 1/(||proj_b|| + eps)
        dump = sm.tile([M, D], f32)  # scratch target for Square outputs
        nrm2 = sm.tile([BPC, 1], f32)
        nc.scalar.activation(dump[0:BPC, :], proj[:], AF.Square, accum_out=nrm2[:])
        nrm = sm.tile([BPC, 1], f32)
        nc.scalar.sqrt(nrm[:], nrm2[:])
        nc.vector.tensor_scalar_add(nrm[:], nrm[:], EPS)
        inv_inp = sm.tile([BPC, 1], f32)
        nc.vector.reciprocal(inv_inp[:], nrm[:])

        # transpose proj -> projT
        projT = sm.tile([128, DCH * BPC], f32)
        for c in range(DCH):
            tp = ps_t.tile([128, BPC], f32, tag="tp")
            nc.tensor.transpose(tp[:], proj[:, c * 128:(c + 1) * 128], ident[0:BPC, 0:BPC])
            nc.vector.tensor_copy(projT[:, c * BPC:(c + 1) * BPC], tp[:])

        # normalize patterns rows, then transpose -> patT [128, DCH*M]
        pn2 = sm.tile([M, 1], f32)
        nc.scalar.activation(dump[:], pat[:], AF.Square, accum_out=pn2[:])
        pnr = sm.tile([M, 1], f32)
        nc.scalar.sqrt(pnr[:], pn2[:])
        nc.vector.tensor_scalar_add(pnr[:], pnr[:], EPS)
        inv_pat = sm.tile([M, 1], f32)
        nc.vector.reciprocal(inv_pat[:], pnr[:])
        patn = sm.tile([M, D], f32)
        nc.scalar.activation(patn[:], pat[:], AF.Copy, scale=inv_pat[:])
        patT = sm.tile([128, DCH * M], f32)
        for c in range(DCH):
            tpp = ps_t.tile([128, M], f32, tag="tp")
            nc.tensor.transpose(tpp[:], patn[:, c * 128:(c + 1) * 128], ident[0:M, 0:M])
            nc.vector.tensor_copy(patT[:, c * M:(c + 1) * M], tpp[:])

        # G[b,m] = proj_b . patn_m   (cosine sim up to the positive 1/|proj_b| factor)
        g_ps = ps_g.tile([BPC, M], f32, tag="g")
        for c in range(DCH):
            nc.tensor.matmul(
                g_ps[:],
                lhsT=projT[:, c * BPC:(c + 1) * BPC],
                rhs=patT[:, c * M:(c + 1) * M],
                start=(c == 0),
                stop=(c == DCH - 1),
            )
        graw = sm.tile([BPC, M], f32)
        nc.scalar.copy(graw[:], g_ps[:])

        # row max + one-hot(argmax); scaling by inv_inp>0 preserves argmax
        rowmax = sm.tile([BPC, 1], f32)
        nc.vector.reduce_max(rowmax[:], graw[:], axis=AX.X)
        oheq = sm.tile([BPC, M], f32)
        nc.vector.tensor_scalar(oheq[:], graw[:], rowmax[:], None, op0=ALU.is_equal)

        # gate = sigmoid(max sim); mask = gate > threshold
        score = sm.tile([BPC, 1], f32)
        nc.scalar.activation(score[:], rowmax[:], AF.Copy, scale=inv_inp[:])
        gate = sm.tile([BPC, 1], f32)
        nc.scalar.activation(gate[:], score[:], AF.Sigmoid)
        mask = sm.tile([BPC, 1], f32)
        nc.vector.tensor_scalar(mask[:], gate[:], THRESHOLD, None, op0=ALU.is_gt)

        # sel = onehot @ bias_table  via PE; then mask rows
        tpo = ps_t.tile([M, BPC], f32, tag="tp")
        nc.tensor.transpose(tpo[:], oheq[:], ident[0:BPC, 0:BPC])
        ohT = sm.tile([M, BPC], f32)
        nc.vector.tensor_copy(ohT[:], tpo[:])
        sel_ps = ps_g.tile([BPC, H], f32, tag="g")
        nc.tensor.matmul(sel_ps[:], lhsT=ohT[:], rhs=bt[:], start=True, stop=True)
        out_sb = sm.tile([BPC, H], f32)
        nc.scalar.activation(out_sb[:], sel_ps[:], AF.Copy, scale=mask[:])
        nc.scalar.dma_start(out_d[:], out_sb[:])

    nc.compile()
    _CACHE["nc"] = nc
    return nc


def make_in_maps(inputs):
    """Shard full inputs into per-core input maps (host-side, views only)."""
    x = np.ascontiguousarray(np.asarray(inputs["x"], dtype=np.float32))
    W = np.ascontiguousarray(np.asarray(inputs["W_p"], dtype=np.float32))
    bp = np.ascontiguousarray(np.asarray(inputs["b_p"], dtype=np.float32))
    pat = np.ascontiguousarray(np.asarray(inputs["patterns"], dtype=np.float32))
    bt = np.ascontiguousarray(np.asarray(inputs["bias_table"], dtype=np.float32))

    oh = np.zeros((128, BPC * BPC), np.float32)
    for b in range(BPC):
        oh[:, b * BPC + b] = 1.0
    ones1 = np.ones((1, BPC), np.float32)
    ident = np.eye(128, dtype=np.float32)
    Wr = W.reshape(DCH, 128, D)
    bp2 = bp.reshape(1, D)

    in_maps = []
    for c in range(NCORES):
        xs = x[c * BPC:(c + 1) * BPC].reshape(BPC, KCH, 128, JD, D)
        in_maps.append({
            "x": xs, "W_p": Wr, "b_p": bp2, "patterns": pat,
            "bias_table": bt, "oh": oh, "ones1": ones1, "ident": ident,
        })
    return in_maps


def kernel(**inputs) -> np.ndarray:
    from concourse.bass_utils import run_bass_kernel_spmd

    nc = build_bass()
    in_maps = make_in_maps(inputs)
    res = run_bass_kernel_spmd(nc, in_maps, list(range(NCORES)))
    return np.concatenate([r["out"] for r in res.results], axis=0)


# revision 13
# speedup vs baseline: 1.0107x; 1.0107x over previous
"""Trainium2 Bass kernel for nn_BiasingGateB (retrieval_knn, 8 NeuronCores).

Reference computation (for x:[64,2048,1024] f32):
    inp  = mean_T(x) @ W_p + b_p                        # [64,1024]
    sim  = cosine_sim(inp, patterns)                    # [64,64]
    gate = sigmoid(max_m sim)
    out  = where(gate > 0.8, bias_table[argmax_m sim], 0)   # [64,16]

Sharding: data-parallel over batch. Core c owns batches [8c, 8c+8).
The 512 MB x tensor dominates (memory regime): each core streams its
64 MB shard through SBUF in 16 x 4 MB chunks and reduces over T on the
TensorEngine (one-hot-column matmuls accumulating into one PSUM
[8,1024] tile), which hides the reduction entirely under the DMA
stream. W_p / patterns / bias_table are replicated. The projection,
normalization, similarity, argmax/gate and bias_table gather run as a
tiny on-device epilogue; the host only shards inputs and concatenates
the eight [8,16] outputs. xa is unused by the reference and never
touched.
"""

import numpy as np

B, T, D, H, M = 64, 2048, 1024, 16, 64
NCORES = 8
BPC = B // NCORES       # batches per core
KCH = 2                 # x-chunks per batch (4 MB each)
JD = T // (KCH * 128)   # T-rows per partition per chunk
DCH = D // 128          # 128-row chunks of the contraction dim
EPS = 1e-8
THRESHOLD = 0.8

_CACHE = {}


def build_bass():
    """Build (and cache) the per-core Bass module."""
    if "nc" in _CACHE:
        return _CACHE["nc"]

    import concourse.bacc as bacc
    import concourse.bass as bass
    import concourse.mybir as mybir
    import concourse.tile as tile
    from contextlib import ExitStack

    f32 = mybir.dt.float32
    f32r = mybir.dt.float32r
    bf16 = mybir.dt.bfloat16
    AF = mybir.ActivationFunctionType
    ALU = mybir.AluOpType
    AX = mybir.AxisListType
    PSUM = bass.MemorySpace.PSUM

    nc = bacc.Bacc("TRN2", target_bir_lowering=False, debug=False)

    x_d = nc.declare_dram_parameter("x", [BPC, KCH, 128, JD, D], f32r, isOutput=False)
    w_d = nc.declare_dram_parameter("W_p", [DCH, 128, D], f32, isOutput=False)
    bp_d = nc.declare_dram_parameter("b_p", [1, D], f32, isOutput=False)
    pat_d = nc.declare_dram_parameter("patterns", [M, D], f32, isOutput=False)
    bt_d = nc.declare_dram_parameter("bias_table", [M, H], f32, isOutput=False)
    oh_d = nc.declare_dram_parameter("oh", [128, BPC * BPC], f32r, isOutput=False)
    one_d = nc.declare_dram_parameter("ones1", [1, BPC], f32, isOutput=False)
    id_d = nc.declare_dram_parameter("ident", [128, 128], f32, isOutput=False)
    out_d = nc.declare_dram_parameter("out", [BPC, H], f32, isOutput=True)

    with tile.TileContext(nc) as tc, ExitStack() as ctx:
        xp = ctx.enter_context(tc.tile_pool(name="xp", bufs=4))
        cst = ctx.enter_context(tc.tile_pool(name="cst", bufs=1))
        sm = ctx.enter_context(tc.tile_pool(name="sm", bufs=1))
        ps_s = ctx.enter_context(tc.tile_pool(name="ps_s", bufs=1, space=PSUM))
        ps_p = ctx.enter_context(tc.tile_pool(name="ps_p", bufs=1, space=PSUM))
        ps_t = ctx.enter_context(tc.tile_pool(name="ps_t", bufs=2, space=PSUM))
        ps_g = ctx.enter_context(tc.tile_pool(name="ps_g", bufs=1, space=PSUM))

        # Small replicated inputs + constants (SWDGE queue, off the x stream)
        oh = cst.tile([128, BPC * BPC], f32r)
        nc.gpsimd.dma_start(oh[:], oh_d[:])
        # (epilogue-only constants are loaded after the stream loop below)

        # ---- Phase 1: stream x, accumulate per-batch sums over T into PSUM.
        # lhsT = one-hot column block for batch b, so row b of s_ps gets
        # sum_k rhs[k,:] and every other row accumulates 0.
        s_ps = ps_s.tile([BPC, D], f32)
        first = (0, 0, 0)
        last = (BPC - 1, KCH - 1, JD - 1)
        # each chunk's two 2 MB halves go to two different queues so the
        # three queues stay load-balanced and drain together
        qpairs = [(nc.sync, nc.scalar), (nc.scalar, nc.gpsimd), (nc.gpsimd, nc.sync)]
        for b in range(BPC):
            for k in range(KCH):
                xt = xp.tile([128, JD, D], f32r)
                e1, e2 = qpairs[(b * KCH + k) % 3]
                half = JD // 2
                e1.dma_start(xt[:, 0:half, :], x_d[b, k, :, 0:half, :])
                e2.dma_start(xt[:, half:JD, :], x_d[b, k, :, half:JD, :])
                for j in range(JD):
                    for h in range(2):
                        nc.tensor.matmul(
                            s_ps[:, h * 512:(h + 1) * 512],
                            lhsT=oh[:, b * BPC:(b + 1) * BPC],
                            rhs=xt[:, j, h * 512:(h + 1) * 512],
                            start=((b, k, j) == first),
                            stop=((b, k, j) == last),
                        )

        ident = cst.tile([128, 128], f32)
        nc.gpsimd.dma_start(ident[:], id_d[:])
        ones1 = cst.tile([1, BPC], bf16)
        nc.gpsimd.dma_start(ones1[:], one_d[:])
        bp = cst.tile([1, D], bf16)
        nc.gpsimd.dma_start(bp[:], bp_d[:])
        pat = cst.tile([M, D], f32)
        nc.gpsimd.dma_start(pat[:], pat_d[:])
        bt = cst.tile([M, H], f32)
        nc.gpsimd.dma_start(bt[:], bt_d[:])
        # W_p cast to bf16 in the SWDGE transfer: halves its SBUF footprint
        # (frees room for a 4th x buffer) and single-pass PE matmuls
        wt = cst.tile([128, DCH, D], bf16)
        for c in range(DCH):
            nc.gpsimd.dma_start(wt[:, c, :], w_d[c])

        # ---- Phase 2 (tiny epilogue, all on-device) ----
        # mean over T
        inp = sm.tile([BPC, D], f32)
        nc.scalar.mul(inp[:], s_ps[:], 1.0 / T)

        # transpose inp -> inpT [128, DCH*BPC] (d-chunk c in cols [c*8,(c+1)*8))
        inpT = sm.tile([128, DCH * BPC], bf16)
        for c in range(DCH):
            tp = ps_t.tile([128, BPC], f32, tag="tp")
            nc.tensor.transpose(tp[:], inp[:, c * 128:(c + 1) * 128], ident[0:BPC, 0:BPC])
            nc.vector.tensor_copy(inpT[:, c * BPC:(c + 1) * BPC], tp[:])

        # proj = inp @ W_p + b_p  (bias folded in as a K=1 matmul)
        p_ps = ps_p.tile([BPC, D], f32)
        for h in range(2):
            for c in range(DCH):
                nc.tensor.matmul(
                    p_ps[:, h * 512:(h + 1) * 512],
                    lhsT=inpT[:, c * BPC:(c + 1) * BPC],
                    rhs=wt[:, c, h * 512:(h + 1) * 512],
                    start=(c == 0),
                    stop=False,
                )
            nc.tensor.matmul(
                p_ps[:, h * 512:(h + 1) * 512],
                lhsT=ones1[0:1, :],
                rhs=bp[0:1, h * 512:(h + 1) * 512],
                start=False,
                stop=True,
            )
        proj = sm.tile([BPC, D], f32)
        nc.scalar.copy(proj[:], p_ps[:])

        # row norms of proj -> inv_inp = 1/(||proj_b|| + eps)
        dump = sm.tile([M, D], f32)  # scratch target for Square outputs
        nrm2 = sm.tile([BPC, 1], f32)
        nc.scalar.activation(dump[0:BPC, :], proj[:], AF.Square, accum_out=nrm2[:])
        nrm = sm.tile([BPC, 1], f32)
        nc.scalar.sqrt(nrm[:], nrm2[:])
        nc.vector.tensor_scalar_add(nrm[:], nrm[:], EPS)
        inv_inp = sm.tile([BPC, 1], f32)
        nc.vector.reciprocal(inv_inp[:], nrm[:])

        # transpose proj -> projT
        projT = sm.tile([128, DCH * BPC], f32)
        for c in range(DCH):
            tp = ps_t.tile([128, BPC], f32, tag="tp")
            nc.tensor.transpose(tp[:], proj[:, c * 128:(c + 1) * 128], ident[0:BPC, 0:BPC])
            nc.vector.tensor_copy(projT[:, c * BPC:(c + 1) * BPC], tp[:])

        # normalize patterns rows, then transpose -> patT [128, DCH*M]
        pn2 = sm.tile([M, 1], f32)
        nc.scalar.activation(dump[:], pat[:], AF.Square, accum_out=pn2[:])
        pnr = sm.tile([M, 1], f32)
        nc.scalar.sqrt(pnr[:], pn2[:])
        nc.vector.tensor_scalar_add(pnr[:], pnr[:], EPS)
        inv_pat = sm.tile([M, 1], f32)
        nc.vector.reciprocal(inv_pat[:], pnr[:])
        patn = sm.tile([M, D], f32)
        nc.scalar.activation(patn[:], pat[:], AF.Copy, scale=inv_pat[:])
        patT = sm.tile([128, DCH * M], f32)
        for c in range(DCH):
            tpp = ps_t.tile([128, M], f32, tag="tp")
            nc.tensor.transpose(tpp[:], patn[:, c * 128:(c + 1) * 128], ident[0:M, 0:M])
            nc.vector.tensor_copy(patT[:, c * M:(c + 1) * M], tpp[:])

        # G[b,m] = proj_b . patn_m   (cosine sim up to the positive 1/|proj_b| factor)
        g_ps = ps_g.tile([BPC, M], f32, tag="g")
        for c in range(DCH):
            nc.tensor.matmul(
                g_ps[:],
                lhsT=projT[:, c * BPC:(c + 1) * BPC],
                rhs=patT[:, c * M:(c + 1) * M],
                start=(c == 0),
                stop=(c == DCH - 1),
            )
        graw = sm.tile([BPC, M], f32)
        nc.scalar.copy(graw[:], g_ps[:])

        # row max + one-hot(argmax); scaling by inv_inp>0 preserves argmax
        rowmax = sm.tile([BPC, 1], f32)
        nc.vector.reduce_max(rowmax[:], graw[:], axis=AX.X)
        oheq = sm.tile([BPC, M], f32)
        nc.vector.tensor_scalar(oheq[:], graw[:], rowmax[:], None, op0=ALU.is_equal)

        # gate = sigmoid(max sim); mask = gate > threshold
        score = sm.tile([BPC, 1], f32)
        nc.scalar.activation(score[:], rowmax[:], AF.Copy, scale=inv_inp[:])
        gate = sm.tile([BPC, 1], f32)
        nc.scalar.activation(gate[:], score[:], AF.Sigmoid)
        mask = sm.tile([BPC, 1], f32)
        nc.vector.tensor_scalar(mask[:], gate[:], THRESHOLD, None, op0=ALU.is_gt)

        # sel = onehot @ bias_table  via PE; then mask rows
        tpo = ps_t.tile([M, BPC], f32, tag="tp")
        nc.tensor.transpose(tpo[:], oheq[:], ident[0:BPC, 0:BPC])
        ohT = sm.tile([M, BPC], f32)
        nc.vector.tensor_copy(ohT[:], tpo[:])
        sel_ps = ps_g.tile([BPC, H], f32, tag="g")
        nc.tensor.matmul(sel_ps[:], lhsT=ohT[:], rhs=bt[:], start=True, stop=True)
        out_sb = sm.tile([BPC, H], f32)
        nc.scalar.activation(out_sb[:], sel_ps[:], AF.Copy, scale=mask[:])
        nc.scalar.dma_start(out_d[:], out_sb[:])

    nc.compile()
    _CACHE["nc"] = nc
    return nc


def make_in_maps(inputs):
    """Shard full inputs into per-core input maps (host-side, views only)."""
    x = np.ascontiguousarray(np.asarray(inputs["x"], dtype=np.float32))
    W = np.ascontiguousarray(np.asarray(inputs["W_p"], dtype=np.float32))
    bp = np.ascontiguousarray(np.asarray(inputs["b_p"], dtype=np.float32))
    pat = np.ascontiguousarray(np.asarray(inputs["patterns"], dtype=np.float32))
    bt = np.ascontiguousarray(np.asarray(inputs["bias_table"], dtype=np.float32))

    oh = np.zeros((128, BPC * BPC), np.float32)
    for b in range(BPC):
        oh[:, b * BPC + b] = 1.0
    ones1 = np.ones((1, BPC), np.float32)
    ident = np.eye(128, dtype=np.float32)
    Wr = W.reshape(DCH, 128, D)
    bp2 = bp.reshape(1, D)

    in_maps = []
    for c in range(NCORES):
        xs = x[c * BPC:(c + 1) * BPC].reshape(BPC, KCH, 128, JD, D)
        in_maps.append({
            "x": xs, "W_p": Wr, "b_p": bp2, "patterns": pat,
            "bias_table": bt, "oh": oh, "ones1": ones1, "ident": ident,
        })
    return in_maps


def kernel(**inputs) -> np.ndarray:
    from concourse.bass_utils import run_bass_kernel_spmd

    nc = build_bass()
    in_maps = make_in_maps(inputs)
    res = run_bass_kernel_spmd(nc, in_maps, list(range(NCORES)))
    return np.concatenate([r["out"] for r in res.results], axis=0)


# revision 14
# speedup vs baseline: 1.0364x; 1.0254x over previous
"""Trainium2 Bass kernel for nn_BiasingGateB (retrieval_knn, 8 NeuronCores).

Reference computation (for x:[64,2048,1024] f32):
    inp  = mean_T(x) @ W_p + b_p                        # [64,1024]
    sim  = cosine_sim(inp, patterns)                    # [64,64]
    gate = sigmoid(max_m sim)
    out  = where(gate > 0.8, bias_table[argmax_m sim], 0)   # [64,16]

Sharding: data-parallel over batch. Core c owns batches [8c, 8c+8).
The 512 MB x tensor dominates (memory regime): each core streams its
64 MB shard through SBUF in 16 x 4 MB chunks and reduces over T on the
TensorEngine (one-hot-column matmuls accumulating into one PSUM
[8,1024] tile), which hides the reduction entirely under the DMA
stream. W_p / patterns / bias_table are replicated. The projection,
normalization, similarity, argmax/gate and bias_table gather run as a
tiny on-device epilogue; the host only shards inputs and concatenates
the eight [8,16] outputs. xa is unused by the reference and never
touched.
"""

import numpy as np

B, T, D, H, M = 64, 2048, 1024, 16, 64
NCORES = 8
BPC = B // NCORES       # batches per core
KCH = 2                 # x-chunks per batch (4 MB each)
JD = T // (KCH * 128)   # T-rows per partition per chunk
DCH = D // 128          # 128-row chunks of the contraction dim
EPS = 1e-8
THRESHOLD = 0.8

_CACHE = {}


def build_bass():
    """Build (and cache) the per-core Bass module."""
    if "nc" in _CACHE:
        return _CACHE["nc"]

    import concourse.bacc as bacc
    import concourse.bass as bass
    import concourse.mybir as mybir
    import concourse.tile as tile
    from contextlib import ExitStack

    f32 = mybir.dt.float32
    f32r = mybir.dt.float32r
    bf16 = mybir.dt.bfloat16
    AF = mybir.ActivationFunctionType
    ALU = mybir.AluOpType
    AX = mybir.AxisListType
    PSUM = bass.MemorySpace.PSUM

    nc = bacc.Bacc("TRN2", target_bir_lowering=False, debug=False)

    x_d = nc.declare_dram_parameter("x", [BPC, KCH, 128, JD, D], f32r, isOutput=False)
    w_d = nc.declare_dram_parameter("W_p", [DCH, 128, D], f32, isOutput=False)
    bp_d = nc.declare_dram_parameter("b_p", [1, D], f32, isOutput=False)
    pat_d = nc.declare_dram_parameter("patterns", [M, D], f32, isOutput=False)
    bt_d = nc.declare_dram_parameter("bias_table", [M, H], f32, isOutput=False)
    oh_d = nc.declare_dram_parameter("oh", [128, BPC * BPC], f32r, isOutput=False)
    one_d = nc.declare_dram_parameter("ones1", [1, BPC], f32, isOutput=False)
    id_d = nc.declare_dram_parameter("ident", [128, 128], f32, isOutput=False)
    out_d = nc.declare_dram_parameter("out", [BPC, H], f32, isOutput=True)

    with tile.TileContext(nc) as tc, ExitStack() as ctx:
        xp = ctx.enter_context(tc.tile_pool(name="xp", bufs=4))
        cst = ctx.enter_context(tc.tile_pool(name="cst", bufs=1))
        sm = ctx.enter_context(tc.tile_pool(name="sm", bufs=1))
        ps_s = ctx.enter_context(tc.tile_pool(name="ps_s", bufs=1, space=PSUM))
        ps_p = ctx.enter_context(tc.tile_pool(name="ps_p", bufs=1, space=PSUM))
        ps_t = ctx.enter_context(tc.tile_pool(name="ps_t", bufs=2, space=PSUM))
        ps_g = ctx.enter_context(tc.tile_pool(name="ps_g", bufs=1, space=PSUM))

        # Small replicated inputs + constants (SWDGE queue, off the x stream)
        oh = cst.tile([128, BPC * BPC], f32r)
        nc.gpsimd.dma_start(oh[:], oh_d[:])
        # (epilogue-only constants are loaded after the stream loop below)

        # ---- Phase 1: stream x, accumulate per-batch sums over T into PSUM.
        # lhsT = one-hot column block for batch b, so row b of s_ps gets
        # sum_k rhs[k,:] and every other row accumulates 0.
        s_ps = ps_s.tile([BPC, D], f32)
        first = (0, 0, 0)
        last = (BPC - 1, KCH - 1, JD - 1)
        # explicit queue schedule: sync/scalar take 24 MB each, gpsimd
        # (which also carries the ~5.6 MB of constants) takes 16 MB
        qsched = [nc.sync, nc.scalar, nc.gpsimd] * 4 + [nc.sync, nc.scalar] * 2
        for b in range(BPC):
            for k in range(KCH):
                xt = xp.tile([128, JD, D], f32r)
                eng = qsched[b * KCH + k]
                if b == 0 and k == 0:
                    # split the first chunk into 1 MB pieces so the PE (and
                    # buffer-slot recycling) starts as early as possible
                    for s_ in range(4):
                        eng.dma_start(xt[:, 2 * s_:2 * s_ + 2, :],
                                      x_d[0, 0, :, 2 * s_:2 * s_ + 2, :])
                else:
                    eng.dma_start(xt[:], x_d[b, k])
                for j in range(JD):
                    for h in range(2):
                        nc.tensor.matmul(
                            s_ps[:, h * 512:(h + 1) * 512],
                            lhsT=oh[:, b * BPC:(b + 1) * BPC],
                            rhs=xt[:, j, h * 512:(h + 1) * 512],
                            start=((b, k, j) == first),
                            stop=((b, k, j) == last),
                        )

        ident = cst.tile([128, 128], f32)
        nc.gpsimd.dma_start(ident[:], id_d[:])
        ones1 = cst.tile([1, BPC], bf16)
        nc.gpsimd.dma_start(ones1[:], one_d[:])
        bp = cst.tile([1, D], bf16)
        nc.gpsimd.dma_start(bp[:], bp_d[:])
        pat = cst.tile([M, D], f32)
        nc.gpsimd.dma_start(pat[:], pat_d[:])
        bt = cst.tile([M, H], f32)
        nc.gpsimd.dma_start(bt[:], bt_d[:])
        # W_p cast to bf16 in the SWDGE transfer: halves its SBUF footprint
        # (frees room for a 4th x buffer) and single-pass PE matmuls
        wt = cst.tile([128, DCH, D], bf16)
        for c in range(DCH):
            nc.gpsimd.dma_start(wt[:, c, :], w_d[c])

        # ---- Phase 2 (tiny epilogue, all on-device) ----
        # mean over T
        inp = sm.tile([BPC, D], f32)
        nc.scalar.mul(inp[:], s_ps[:], 1.0 / T)

        # transpose inp -> inpT [128, DCH*BPC] (d-chunk c in cols [c*8,(c+1)*8))
        inpT = sm.tile([128, DCH * BPC], bf16)
        for c in range(DCH):
            tp = ps_t.tile([128, BPC], f32, tag="tp")
            nc.tensor.transpose(tp[:], inp[:, c * 128:(c + 1) * 128], ident[0:BPC, 0:BPC])
            nc.vector.tensor_copy(inpT[:, c * BPC:(c + 1) * BPC], tp[:])

        # proj = inp @ W_p + b_p  (bias folded in as a K=1 matmul)
        p_ps = ps_p.tile([BPC, D], f32)
        for h in range(2):
            for c in range(DCH):
                nc.tensor.matmul(
                    p_ps[:, h * 512:(h + 1) * 512],
                    lhsT=inpT[:, c * BPC:(c + 1) * BPC],
                    rhs=wt[:, c, h * 512:(h + 1) * 512],
                    start=(c == 0),
                    stop=False,
                )
            nc.tensor.matmul(
                p_ps[:, h * 512:(h + 1) * 512],
                lhsT=ones1[0:1, :],
                rhs=bp[0:1, h * 512:(h + 1) * 512],
                start=False,
                stop=True,
            )
        proj = sm.tile([BPC, D], f32)
        nc.scalar.copy(proj[:], p_ps[:])

        # row norms of proj -> inv_inp = 1/(||proj_b|| + eps)
        dump = sm.tile([M, D], f32)  # scratch target for Square outputs
        nrm2 = sm.tile([BPC, 1], f32)
        nc.scalar.activation(dump[0:BPC, :], proj[:], AF.Square, accum_out=nrm2[:])
        nrm = sm.tile([BPC, 1], f32)
        nc.scalar.sqrt(nrm[:], nrm2[:])
        nc.vector.tensor_scalar_add(nrm[:], nrm[:], EPS)
        inv_inp = sm.tile([BPC, 1], f32)
        nc.vector.reciprocal(inv_inp[:], nrm[:])

        # transpose proj -> projT
        projT = sm.tile([128, DCH * BPC], f32)
        for c in range(DCH):
            tp = ps_t.tile([128, BPC], f32, tag="tp")
            nc.tensor.transpose(tp[:], proj[:, c * 128:(c + 1) * 128], ident[0:BPC, 0:BPC])
            nc.vector.tensor_copy(projT[:, c * BPC:(c + 1) * BPC], tp[:])

        # normalize patterns rows, then transpose -> patT [128, DCH*M]
        pn2 = sm.tile([M, 1], f32)
        nc.scalar.activation(dump[:], pat[:], AF.Square, accum_out=pn2[:])
        pnr = sm.tile([M, 1], f32)
        nc.scalar.sqrt(pnr[:], pn2[:])
        nc.vector.tensor_scalar_add(pnr[:], pnr[:], EPS)
        inv_pat = sm.tile([M, 1], f32)
        nc.vector.reciprocal(inv_pat[:], pnr[:])
        patn = sm.tile([M, D], f32)
        nc.scalar.activation(patn[:], pat[:], AF.Copy, scale=inv_pat[:])
        patT = sm.tile([128, DCH * M], f32)
        for c in range(DCH):
            tpp = ps_t.tile([128, M], f32, tag="tp")
            nc.tensor.transpose(tpp[:], patn[:, c * 128:(c + 1) * 128], ident[0:M, 0:M])
            nc.vector.tensor_copy(patT[:, c * M:(c + 1) * M], tpp[:])

        # G[b,m] = proj_b . patn_m   (cosine sim up to the positive 1/|proj_b| factor)
        g_ps = ps_g.tile([BPC, M], f32, tag="g")
        for c in range(DCH):
            nc.tensor.matmul(
                g_ps[:],
                lhsT=projT[:, c * BPC:(c + 1) * BPC],
                rhs=patT[:, c * M:(c + 1) * M],
                start=(c == 0),
                stop=(c == DCH - 1),
            )
        graw = sm.tile([BPC, M], f32)
        nc.scalar.copy(graw[:], g_ps[:])

        # row max + one-hot(argmax); scaling by inv_inp>0 preserves argmax
        rowmax = sm.tile([BPC, 1], f32)
        nc.vector.reduce_max(rowmax[:], graw[:], axis=AX.X)
        oheq = sm.tile([BPC, M], f32)
        nc.vector.tensor_scalar(oheq[:], graw[:], rowmax[:], None, op0=ALU.is_equal)

        # gate = sigmoid(max sim); mask = gate > threshold
        score = sm.tile([BPC, 1], f32)
        nc.scalar.activation(score[:], rowmax[:], AF.Copy, scale=inv_inp[:])
        gate = sm.tile([BPC, 1], f32)
        nc.scalar.activation(gate[:], score[:], AF.Sigmoid)
        mask = sm.tile([BPC, 1], f32)
        nc.vector.tensor_scalar(mask[:], gate[:], THRESHOLD, None, op0=ALU.is_gt)

        # sel = onehot @ bias_table  via PE; then mask rows
        tpo = ps_t.tile([M, BPC], f32, tag="tp")
        nc.tensor.transpose(tpo[:], oheq[:], ident[0:BPC, 0:BPC])
        ohT = sm.tile([M, BPC], f32)
        nc.vector.tensor_copy(ohT[:], tpo[:])
        sel_ps = ps_g.tile([BPC, H], f32, tag="g")
        nc.tensor.matmul(sel_ps[:], lhsT=ohT[:], rhs=bt[:], start=True, stop=True)
        out_sb = sm.tile([BPC, H], f32)
        nc.scalar.activation(out_sb[:], sel_ps[:], AF.Copy, scale=mask[:])
        nc.sync.dma_start(out_d[:], out_sb[:])

    nc.compile()
    _CACHE["nc"] = nc
    return nc


def make_in_maps(inputs):
    """Shard full inputs into per-core input maps (host-side, views only)."""
    x = np.ascontiguousarray(np.asarray(inputs["x"], dtype=np.float32))
    W = np.ascontiguousarray(np.asarray(inputs["W_p"], dtype=np.float32))
    bp = np.ascontiguousarray(np.asarray(inputs["b_p"], dtype=np.float32))
    pat = np.ascontiguousarray(np.asarray(inputs["patterns"], dtype=np.float32))
    bt = np.ascontiguousarray(np.asarray(inputs["bias_table"], dtype=np.float32))

    oh = np.zeros((128, BPC * BPC), np.float32)
    for b in range(BPC):
        oh[:, b * BPC + b] = 1.0
    ones1 = np.ones((1, BPC), np.float32)
    ident = np.eye(128, dtype=np.float32)
    Wr = W.reshape(DCH, 128, D)
    bp2 = bp.reshape(1, D)

    in_maps = []
    for c in range(NCORES):
        xs = x[c * BPC:(c + 1) * BPC].reshape(BPC, KCH, 128, JD, D)
        in_maps.append({
            "x": xs, "W_p": Wr, "b_p": bp2, "patterns": pat,
            "bias_table": bt, "oh": oh, "ones1": ones1, "ident": ident,
        })
    return in_maps


def kernel(**inputs) -> np.ndarray:
    from concourse.bass_utils import run_bass_kernel_spmd

    nc = build_bass()
    in_maps = make_in_maps(inputs)
    res = run_bass_kernel_spmd(nc, in_maps, list(range(NCORES)))
    return np.concatenate([r["out"] for r in res.results], axis=0)


# revision 15
# speedup vs baseline: 1.1357x; 1.0958x over previous
"""Trainium2 Bass kernel for nn_BiasingGateB (retrieval_knn, 8 NeuronCores).

Reference computation (for x:[64,2048,1024] f32):
    inp  = mean_T(x) @ W_p + b_p                        # [64,1024]
    sim  = cosine_sim(inp, patterns)                    # [64,64]
    gate = sigmoid(max_m sim)
    out  = where(gate > 0.8, bias_table[argmax_m sim], 0)   # [64,16]

Sharding: data-parallel over batch. Core c owns batches [8c, 8c+8).
The 512 MB x tensor dominates (memory regime): each core streams its
64 MB shard through SBUF in 16 x 4 MB chunks and reduces over T on the
TensorEngine (one-hot-column matmuls accumulating into one PSUM
[8,1024] tile), which hides the reduction entirely under the DMA
stream. W_p / patterns / bias_table are replicated. The projection,
normalization, similarity, argmax/gate and bias_table gather run as a
tiny on-device epilogue; the host only shards inputs and concatenates
the eight [8,16] outputs. xa is unused by the reference and never
touched.
"""

import numpy as np

B, T, D, H, M = 64, 2048, 1024, 16, 64
NCORES = 8
BPC = B // NCORES       # batches per core
KCH = 4                 # x-chunks per batch (2 MB each)
JD = T // (KCH * 128)   # T-rows per partition per chunk
DCH = D // 128          # 128-row chunks of the contraction dim
EPS = 1e-8
THRESHOLD = 0.8

_CACHE = {}


def build_bass():
    """Build (and cache) the per-core Bass module."""
    if "nc" in _CACHE:
        return _CACHE["nc"]

    import concourse.bacc as bacc
    import concourse.bass as bass
    import concourse.mybir as mybir
    import concourse.tile as tile
    from contextlib import ExitStack

    f32 = mybir.dt.float32
    f32r = mybir.dt.float32r
    bf16 = mybir.dt.bfloat16
    AF = mybir.ActivationFunctionType
    ALU = mybir.AluOpType
    AX = mybir.AxisListType
    PSUM = bass.MemorySpace.PSUM

    nc = bacc.Bacc("TRN2", target_bir_lowering=False, debug=False)

    x_d = nc.declare_dram_parameter("x", [BPC, KCH, 128, JD, D], f32r, isOutput=False)
    w_d = nc.declare_dram_parameter("W_p", [DCH, 128, D], f32, isOutput=False)
    bp_d = nc.declare_dram_parameter("b_p", [1, D], f32, isOutput=False)
    pat_d = nc.declare_dram_parameter("patterns", [M, D], f32, isOutput=False)
    bt_d = nc.declare_dram_parameter("bias_table", [M, H], f32, isOutput=False)
    oh_d = nc.declare_dram_parameter("oh", [128, BPC * BPC], f32r, isOutput=False)
    one_d = nc.declare_dram_parameter("ones1", [1, BPC], f32, isOutput=False)
    id_d = nc.declare_dram_parameter("ident", [128, 128], f32, isOutput=False)
    out_d = nc.declare_dram_parameter("out", [BPC, H], f32, isOutput=True)

    with tile.TileContext(nc) as tc, ExitStack() as ctx:
        xps = ctx.enter_context(tc.tile_pool(name="xps", bufs=3))
        xpc = ctx.enter_context(tc.tile_pool(name="xpc", bufs=3))
        xpg = ctx.enter_context(tc.tile_pool(name="xpg", bufs=2))
        cst = ctx.enter_context(tc.tile_pool(name="cst", bufs=1))
        sm = ctx.enter_context(tc.tile_pool(name="sm", bufs=1))
        ps_s = ctx.enter_context(tc.tile_pool(name="ps_s", bufs=1, space=PSUM))
        ps_p = ctx.enter_context(tc.tile_pool(name="ps_p", bufs=1, space=PSUM))
        ps_t = ctx.enter_context(tc.tile_pool(name="ps_t", bufs=2, space=PSUM))
        ps_g = ctx.enter_context(tc.tile_pool(name="ps_g", bufs=1, space=PSUM))

        # Small replicated inputs + constants (SWDGE queue, off the x stream)
        oh = cst.tile([128, BPC * BPC], f32r)
        nc.gpsimd.dma_start(oh[:], oh_d[:])
        # (epilogue-only constants are loaded after the stream loop below)

        # ---- Phase 1: stream x, accumulate per-batch sums over T into PSUM.
        # lhsT = one-hot column block for batch b, so row b of s_ps gets
        # sum_k rhs[k,:] and every other row accumulates 0.
        s_ps = ps_s.tile([BPC, D], f32)
        first = (0, 0, 0)
        last = (BPC - 1, KCH - 1, JD - 1)
        # Each queue gets a PRIVATE slot pool so one queue's slot
        # recycling never gates another queue's transfers. gpsimd (which
        # also carries ~5.6 MB of constants) gets every 5th chunk; the
        # rest alternate sync/scalar (13 chunks = 26 MB each).
        sc = [0, 1]
        def pick(m):
            if m % 5 == 4:
                return nc.gpsimd, xpg, "xg"
            e = sc[0] % 2
            sc[0] += 1
            return (nc.sync, xps, "xs") if e == 0 else (nc.scalar, xpc, "xc")
        for b in range(BPC):
            for k in range(KCH):
                eng, pool, tg = pick(b * KCH + k)
                xt = pool.tile([128, JD, D], f32r, tag=tg)
                eng.dma_start(xt[:], x_d[b, k])
                for j in range(JD):
                    for h in range(2):
                        nc.tensor.matmul(
                            s_ps[:, h * 512:(h + 1) * 512],
                            lhsT=oh[:, b * BPC:(b + 1) * BPC],
                            rhs=xt[:, j, h * 512:(h + 1) * 512],
                            start=((b, k, j) == first),
                            stop=((b, k, j) == last),
                        )

        ident = cst.tile([128, 128], f32)
        nc.gpsimd.dma_start(ident[:], id_d[:])
        ones1 = cst.tile([1, BPC], bf16)
        nc.gpsimd.dma_start(ones1[:], one_d[:])
        bp = cst.tile([1, D], bf16)
        nc.gpsimd.dma_start(bp[:], bp_d[:])
        pat = cst.tile([M, D], f32)
        nc.gpsimd.dma_start(pat[:], pat_d[:])
        bt = cst.tile([M, H], f32)
        nc.gpsimd.dma_start(bt[:], bt_d[:])
        # W_p cast to bf16 in the SWDGE transfer: halves its SBUF footprint
        # (frees room for a 4th x buffer) and single-pass PE matmuls
        wt = cst.tile([128, DCH, D], bf16)
        for c in range(DCH):
            nc.gpsimd.dma_start(wt[:, c, :], w_d[c])

        # ---- Phase 2 (tiny epilogue, all on-device) ----
        # mean over T
        inp = sm.tile([BPC, D], f32)
        nc.scalar.mul(inp[:], s_ps[:], 1.0 / T)

        # transpose inp -> inpT [128, DCH*BPC] (d-chunk c in cols [c*8,(c+1)*8))
        inpT = sm.tile([128, DCH * BPC], bf16)
        for c in range(DCH):
            tp = ps_t.tile([128, BPC], f32, tag="tp")
            nc.tensor.transpose(tp[:], inp[:, c * 128:(c + 1) * 128], ident[0:BPC, 0:BPC])
            nc.vector.tensor_copy(inpT[:, c * BPC:(c + 1) * BPC], tp[:])

        # proj = inp @ W_p + b_p  (bias folded in as a K=1 matmul)
        p_ps = ps_p.tile([BPC, D], f32)
        for h in range(2):
            for c in range(DCH):
                nc.tensor.matmul(
                    p_ps[:, h * 512:(h + 1) * 512],
                    lhsT=inpT[:, c * BPC:(c + 1) * BPC],
                    rhs=wt[:, c, h * 512:(h + 1) * 512],
                    start=(c == 0),
                    stop=False,
                )
            nc.tensor.matmul(
                p_ps[:, h * 512:(h + 1) * 512],
                lhsT=ones1[0:1, :],
                rhs=bp[0:1, h * 512:(h + 1) * 512],
                start=False,
                stop=True,
            )
        proj = sm.tile([BPC, D], f32)
        nc.scalar.copy(proj[:], p_ps[:])

        # row norms of proj -> inv_inp = 1/(||proj_b|| + eps)
        dump = sm.tile([M, D], f32)  # scratch target for Square outputs
        nrm2 = sm.tile([BPC, 1], f32)
        nc.scalar.activation(dump[0:BPC, :], proj[:], AF.Square, accum_out=nrm2[:])
        nrm = sm.tile([BPC, 1], f32)
        nc.scalar.sqrt(nrm[:], nrm2[:])
        nc.vector.tensor_scalar_add(nrm[:], nrm[:], EPS)
        inv_inp = sm.tile([BPC, 1], f32)
        nc.vector.reciprocal(inv_inp[:], nrm[:])

        # transpose proj -> projT
        projT = sm.tile([128, DCH * BPC], f32)
        for c in range(DCH):
            tp = ps_t.tile([128, BPC], f32, tag="tp")
            nc.tensor.transpose(tp[:], proj[:, c * 128:(c + 1) * 128], ident[0:BPC, 0:BPC])
            nc.vector.tensor_copy(projT[:, c * BPC:(c + 1) * BPC], tp[:])

        # normalize patterns rows, then transpose -> patT [128, DCH*M]
        pn2 = sm.tile([M, 1], f32)
        nc.scalar.activation(dump[:], pat[:], AF.Square, accum_out=pn2[:])
        pnr = sm.tile([M, 1], f32)
        nc.scalar.sqrt(pnr[:], pn2[:])
        nc.vector.tensor_scalar_add(pnr[:], pnr[:], EPS)
        inv_pat = sm.tile([M, 1], f32)
        nc.vector.reciprocal(inv_pat[:], pnr[:])
        patn = sm.tile([M, D], f32)
        nc.scalar.activation(patn[:], pat[:], AF.Copy, scale=inv_pat[:])
        patT = sm.tile([128, DCH * M], f32)
        for c in range(DCH):
            tpp = ps_t.tile([128, M], f32, tag="tp")
            nc.tensor.transpose(tpp[:], patn[:, c * 128:(c + 1) * 128], ident[0:M, 0:M])
            nc.vector.tensor_copy(patT[:, c * M:(c + 1) * M], tpp[:])

        # G[b,m] = proj_b . patn_m   (cosine sim up to the positive 1/|proj_b| factor)
        g_ps = ps_g.tile([BPC, M], f32, tag="g")
        for c in range(DCH):
            nc.tensor.matmul(
                g_ps[:],
                lhsT=projT[:, c * BPC:(c + 1) * BPC],
                rhs=patT[:, c * M:(c + 1) * M],
                start=(c == 0),
                stop=(c == DCH - 1),
            )
        graw = sm.tile([BPC, M], f32)
        nc.scalar.copy(graw[:], g_ps[:])

        # row max + one-hot(argmax); scaling by inv_inp>0 preserves argmax
        rowmax = sm.tile([BPC, 1], f32)
        nc.vector.reduce_max(rowmax[:], graw[:], axis=AX.X)
        oheq = sm.tile([BPC, M], f32)
        nc.vector.tensor_scalar(oheq[:], graw[:], rowmax[:], None, op0=ALU.is_equal)

        # gate = sigmoid(max sim); mask = gate > threshold
        score = sm.tile([BPC, 1], f32)
        nc.scalar.activation(score[:], rowmax[:], AF.Copy, scale=inv_inp[:])
        gate = sm.tile([BPC, 1], f32)
        nc.scalar.activation(gate[:], score[:], AF.Sigmoid)
        mask = sm.tile([BPC, 1], f32)
        nc.vector.tensor_scalar(mask[:], gate[:], THRESHOLD, None, op0=ALU.is_gt)

        # sel = onehot @ bias_table  via PE; then mask rows
        tpo = ps_t.tile([M, BPC], f32, tag="tp")
        nc.tensor.transpose(tpo[:], oheq[:], ident[0:BPC, 0:BPC])
        ohT = sm.tile([M, BPC], f32)
        nc.vector.tensor_copy(ohT[:], tpo[:])
        sel_ps = ps_g.tile([BPC, H], f32, tag="g")
        nc.tensor.matmul(sel_ps[:], lhsT=ohT[:], rhs=bt[:], start=True, stop=True)
        out_sb = sm.tile([BPC, H], f32)
        nc.scalar.activation(out_sb[:], sel_ps[:], AF.Copy, scale=mask[:])
        nc.sync.dma_start(out_d[:], out_sb[:])

    nc.compile()
    _CACHE["nc"] = nc
    return nc


def make_in_maps(inputs):
    """Shard full inputs into per-core input maps (host-side, views only)."""
    x = np.ascontiguousarray(np.asarray(inputs["x"], dtype=np.float32))
    W = np.ascontiguousarray(np.asarray(inputs["W_p"], dtype=np.float32))
    bp = np.ascontiguousarray(np.asarray(inputs["b_p"], dtype=np.float32))
    pat = np.ascontiguousarray(np.asarray(inputs["patterns"], dtype=np.float32))
    bt = np.ascontiguousarray(np.asarray(inputs["bias_table"], dtype=np.float32))

    oh = np.zeros((128, BPC * BPC), np.float32)
    for b in range(BPC):
        oh[:, b * BPC + b] = 1.0
    ones1 = np.ones((1, BPC), np.float32)
    ident = np.eye(128, dtype=np.float32)
    Wr = W.reshape(DCH, 128, D)
    bp2 = bp.reshape(1, D)

    in_maps = []
    for c in range(NCORES):
        xs = x[c * BPC:(c + 1) * BPC].reshape(BPC, KCH, 128, JD, D)
        in_maps.append({
            "x": xs, "W_p": Wr, "b_p": bp2, "patterns": pat,
            "bias_table": bt, "oh": oh, "ones1": ones1, "ident": ident,
        })
    return in_maps


def kernel(**inputs) -> np.ndarray:
    from concourse.bass_utils import run_bass_kernel_spmd

    nc = build_bass()
    in_maps = make_in_maps(inputs)
    res = run_bass_kernel_spmd(nc, in_maps, list(range(NCORES)))
    return np.concatenate([r["out"] for r in res.results], axis=0)
